# revision 36
# baseline (speedup 1.0000x reference)
"""Data-parallel Trainium2 kernel for the weighted classification loss.

loss = -mean_b sum_c w[b,c] * log(1 - softmax(reps @ W.T + b)[b,c])

Strategy (8 cores, batch-sharded 4096 rows each):
  - Host pre-casts reps to fp8e4 and pre-transposes into a matmul-ready
    [k-chunk x sample] layout; the kernel streams it HBM->SBUF with
    plain HWDGE DMAs (no on-chip cast/transpose).
  - Tapered chunks ([1024,1024,1024,512,256,256] samples): each chunk
    is an independent round whose matmuls start on its own DMA
    semaphore; the small final chunks shrink the post-stream tail.
  - Main matmul per chunk: K=128 fp8 chains over 8 D-chunks, 4-way
    column-tiled (tile_position=(0,32j), k-outer/j-inner) so 4
    sample-quarters accumulate concurrently into one PSUM tile as
    logits rows 32j..32j+9.
  - exp(l + bias) on ACT over the whole [128, Q] tile (4 groups at
    once); one diagonal-packed matmul vs a (ones - I | ones)-style
    stationary computes u_c = den - e_c (sum of positives) and den for
    all 4 groups; Ln on ACT; a host-prepared per-sample weight mask
    {0,1,2,-14} contracts w * ln(u) - 14*ln(den) via one DVE
    scalar_tensor_tensor with free-dim accumulate per chunk.
  - Per-chunk partial-sum columns DMA out as they finish (only the
    last sits on the critical tail); host combines.
"""

import os
import sys

import numpy as np

if "/opt/trn_rl_repo" not in sys.path:
    sys.path.insert(0, "/opt/trn_rl_repo")

import ml_dtypes

B, D, C = 32768, 1024, 10
NCORES = 8
SHARD = B // NCORES  # 4096
KCH = D // 128       # 8 contraction chunks
NGRP = 4
# chunk sizes in samples; each is one PSUM round of 4 column-groups.
# The PE FIFO costs ~27ns/instruction, so a 4-group chunk below ~900
# samples is issue-bound (~1.7us floor). Big data-bound chunks minimize
# total PE work; the small LAST chunk runs 2 column-groups (q=128,
# data-bound ~1.1us) to shorten the post-stream tail chain.
CHUNKS = [960, 960, 960, 960, 256]
GRPS = [4, 4, 4, 4, 2]  # column groups per chunk (128//g row stride)
assert sum(CHUNKS) == SHARD and all(
    s % g == 0 for s, g in zip(CHUNKS, GRPS)
)
OFFS = [sum(CHUNKS[:i]) for i in range(len(CHUNKS))]
NCHK = len(CHUNKS)
# mask column offsets: chunk r occupies CHUNKS[r]//GRPS[r] mask columns
MOFFS = [sum(s // g for s, g in zip(CHUNKS[:i], GRPS[:i]))
         for i in range(NCHK)]
MASKW = sum(s // g for s, g in zip(CHUNKS, GRPS))
MID = 5
OPP_W = 2.0

_CACHE: dict = {}


def _build_nc():
    from contextlib import ExitStack

    import concourse.mybir as mybir
    import concourse.tile as tile
    from concourse import bacc
    from concourse.tile import add_dep_helper

    f32 = mybir.dt.float32
    bf16 = mybir.dt.bfloat16
    fp8 = mybir.dt.float8e4
    Exp = mybir.ActivationFunctionType.Exp
    Ln = mybir.ActivationFunctionType.Ln
    alu = mybir.AluOpType

    nc = bacc.Bacc(
        "TRN2",
        target_bir_lowering=False,
        debug=False,
        enable_asserts=False,
        num_devices=NCORES,
    )
    u8 = mybir.dt.uint8
    # The small consts block (uzw4 | uzw64 | wq | bias) is PREPENDED to
    # chunk0 inside the repsq tensor, so one DMA (one ~0.7us SP issue, one
    # semaphore) carries consts+chunk0 — all later chunk issues and
    # completions shift earlier by an issue slot. The big per-sample
    # weight mask rides BEHIND the last input chunk as its own DMA: every
    # chunk's completion shifts earlier by the mask's stream time, and
    # the mask still lands before the STTs need it (they trail the last
    # chunk's matmul+EXP+LN chain anyway).
    UZW_B = 2 * (32 + 64)
    WQ_B = KCH * C
    CONST_B = UZW_B + WQ_B + 4
    MASK_B = 2 * MASKW
    repsq = nc.dram_tensor("repsq", [128, CONST_B + KCH * SHARD], u8,
                           kind="ExternalInput").ap()
    maskc = nc.dram_tensor("maskc", [128, MASK_B], u8,
                           kind="ExternalInput").ap()
    partials = nc.dram_tensor("partials", [128, NCHK], f32,
                              kind="ExternalOutput").ap()

    with tile.TileContext(nc) as tc:
        with ExitStack() as ctx:
            const_pool = ctx.enter_context(tc.tile_pool(name="const", bufs=1))
            sb_pool = ctx.enter_context(tc.tile_pool(name="sb", bufs=3))
            lp_pool = ctx.enter_context(
                tc.tile_pool(name="lp", bufs=3, space="PSUM"))
            u_pool = ctx.enter_context(
                tc.tile_pool(name="u", bufs=2, space="PSUM"))

            # Pin the combined exp+ln activation table (set 6:
            # natural_log_exp_and_others) once, up front, so the compiler's
            # per-function table placement doesn't ping-pong reloads.
            ld_tab = nc.scalar.add_instruction(
                mybir.InstLoadActFuncSet(
                    name=nc.get_next_instruction_name(),
                    ins=[],
                    outs=[],
                    act_func_set_id=6,
                )
            )

            # input chunks on the SP ring (FIFO => chunk c completes at
            # ~its share of the stream); chunk0's DMA also carries the
            # small consts block as a byte prefix. Per-chunk tiles keep
            # Tile's DMA->matmul deps per-chunk.
            xb = []
            cn_t = None
            for c, (off, sz) in enumerate(zip(OFFS, CHUNKS)):
                pre = CONST_B if c == 0 else 0
                t = const_pool.tile([128, pre + KCH * sz], u8, tag=f"x{c}")
                nc.sync.dma_start(
                    t[:],
                    repsq[:, CONST_B + KCH * off - pre
                          : CONST_B + KCH * (off + sz)],
                )
                if c == 0:
                    cn_t = t
                xb.append(
                    t[:, pre:].bitcast(fp8).rearrange(
                        "p (k m) -> p k m", k=KCH)
                )

            # the big weight mask streams BEHIND the last input chunk
            mk_t = const_pool.tile([128, MASK_B], u8, tag="mask")
            nc.sync.dma_start(mk_t[:], maskc)

            uz_all = cn_t[:, 0:UZW_B].bitcast(bf16)
            uzw_t = uz_all[:, 0:32]
            uzw64_t = uz_all[:, 32:96]
            wq_t = cn_t[:, UZW_B : UZW_B + WQ_B].bitcast(fp8)
            bias_t = cn_t[:, UZW_B + WQ_B : CONST_B].bitcast(f32)
            mask_t = mk_t[:].bitcast(bf16)
            acc = const_pool.tile([128, NCHK], f32, tag="acc")
            wv = wq_t.rearrange("p (k c) -> p k c", k=KCH)

            lp_tiles = {}
            first_act = None

            def emit_mains(r):
                g = GRPS[r]
                gw = 128 // g
                q = CHUNKS[r] // g
                lp = lp_pool.tile([128, q], f32, tag="lp", name=f"lp{r}")
                lp_tiles[r] = lp
                # k-outer / j-inner: adjacent MMs hit different col-groups
                # so all g stream concurrently (MATMUL issue is strict FIFO)
                for k in range(KCH):
                    for j in range(g):
                        nc.tensor.matmul(
                            lp[gw * j : gw * j + C, :],
                            wv[:, k, :],
                            xb[r][:, k, j * q : (j + 1) * q],
                            start=(k == 0),
                            stop=(k == KCH - 1),
                            skip_group_check=True,
                            tile_position=(0, gw * j),
                        )

            def emit_tail(r):
                nonlocal first_act
                g = GRPS[r]
                gw = 128 // g
                q = CHUNKS[r] // g
                moff = MOFFS[r]
                lp = lp_tiles.pop(r)
                e = sb_pool.tile([128, q], bf16, tag="e", name=f"e{r}")
                act = nc.scalar.activation(
                    e[:], lp[:], Exp, bias=bias_t, scale=1.0
                )
                if first_act is None:
                    first_act = act
                    add_dep_helper(
                        act.ins, ld_tab.ins, sync=False,
                        reason="combined exp+ln table pinned before first ACT",
                    )

                # u covers ALL 128 partitions (gw-wide stationary) so the
                # Ln below never reads stale PSUM (0 * NaN = NaN in STT)
                u = u_pool.tile([128, q], f32, tag="u", name=f"u{r}")
                uz = uzw_t if g == 4 else uzw64_t
                for j in range(g):
                    nc.tensor.matmul(
                        u[gw * j : gw * j + gw, :],
                        uz[gw * j : gw * j + C, :],
                        e[gw * j : gw * j + C, :],
                        start=True,
                        stop=True,
                        skip_group_check=True,
                        tile_position=(gw * j, gw * j),
                    )

                lnu = sb_pool.tile([128, q], bf16, tag="lnu", name=f"ln{r}")
                nc.scalar.activation(lnu[:], u[:], Ln)

                scr = sb_pool.tile([128, q], f32, tag="scr", name=f"sc{r}")
                nc.vector.scalar_tensor_tensor(
                    out=scr[:],
                    in0=mask_t[:, moff : moff + q],
                    scalar=1.0,
                    in1=lnu[:],
                    op0=alu.mult,
                    op1=alu.mult,
                    accum_out=acc[:, r : r + 1],
                )

            # software-pipelined: round r's tail is emitted after round
            # r+1's matmuls so the PE never stalls waiting on ACT
            emit_mains(0)
            for r in range(1, NCHK):
                emit_mains(r)
                emit_tail(r - 1)
            emit_tail(NCHK - 1)

            # one output DMA for the whole accumulator: splitting it in two
            # backfires — the SP engine is still mid-issue on the first
            # when the last column's semaphore fires, so the single DMA's
            # issue actually starts (and lands) earlier
            nc.sync.dma_start(partials, acc[:])

    nc.compile()
    return nc


def _prepare_static(W: np.ndarray, b: np.ndarray):
    # wq[p, k*C + c] = fp8(W[c, 128k + p])
    wq = np.zeros((128, KCH * C), dtype=np.float32)
    for k in range(KCH):
        wq[:, k * C : (k + 1) * C] = W[:, k * 128 : (k + 1) * 128].T
    wq = wq.astype(ml_dtypes.float8_e4m3)

    # u = uzw_ext.T @ e per group: cols 0..9 -> den - e_c (sum of
    # positives), cols 10.. -> den (keeps every PSUM row defined > 0)
    def uzw_block(width):
        ext = np.ones((C, width), dtype=np.float32)
        ext[:, :C] -= np.eye(C, dtype=np.float32)
        blk = np.zeros((128, width), dtype=np.float32)
        for j in range(128 // width):
            blk[width * j : width * j + C, :] = ext
        return blk

    uzw4 = uzw_block(32)
    uzw64 = uzw_block(64)

    bias4 = np.zeros((128, 1), dtype=np.float32)
    for j in range(NGRP):
        bias4[32 * j : 32 * j + C, 0] = b
    return wq, uzw4, uzw64, bias4


def _prepare_mask(labels_sh: np.ndarray) -> np.ndarray:
    """Per-sample weight mask, bf16 [128, MASKW] viewed as uint8. Layout
    mirrors the on-chip per-chunk groups: chunk r, group j, n -> sample
    OFFS[r] + j*q + n at mask[gw*j + c, MOFFS[r] + n]."""
    cc = np.arange(C).reshape(1, C)
    m = np.zeros((128, MASKW), dtype=np.float32)
    for r, (off, sz) in enumerate(zip(OFFS, CHUNKS)):
        g = GRPS[r]
        gw = 128 // g
        q = sz // g
        moff = MOFFS[r]
        for j in range(g):
            lab = labels_sh[off + j * q : off + (j + 1) * q].astype(np.int64)
            ll = lab.reshape(q, 1)
            opp = (cc < MID) != (ll < MID)
            w = np.where(cc == ll, 0.0, np.where(opp, OPP_W, 1.0))  # [q, C]
            m[gw * j : gw * j + C, moff : moff + q] = w.T
            m[gw * j + C, moff : moff + q] = -float(C + MID - 1)
    return np.ascontiguousarray(m.astype(ml_dtypes.bfloat16)).view(np.uint8)


def _pack_consts(uzw4, uzw64, wq_fp8, bias4_f32) -> np.ndarray:
    """One [128, UZW_B+WQ_B+4] uint8 tensor: uzw4 | uzw64 | wq | bias4."""
    out = np.concatenate(
        [
            np.ascontiguousarray(uzw4.astype(ml_dtypes.bfloat16)).view(
                np.uint8),
            np.ascontiguousarray(uzw64.astype(ml_dtypes.bfloat16)).view(
                np.uint8),
            np.ascontiguousarray(wq_fp8).view(np.uint8),
            np.ascontiguousarray(bias4_f32).view(np.uint8),
        ],
        axis=1,
    )
    return out


def _prepare_reps(reps_sh: np.ndarray) -> np.ndarray:
    """repsq[p, KCH*off + k*sz + m] = fp8(reps_sh[off + m, 128k + p])
    for each chunk (off, sz)."""
    out = np.empty((128, KCH * SHARD), dtype=ml_dtypes.float8_e4m3)
    for off, sz in zip(OFFS, CHUNKS):
        x = reps_sh[off : off + sz].astype(ml_dtypes.float8_e4m3)
        x = x.reshape(sz, KCH, 128)                 # [m, k, p]
        x = np.ascontiguousarray(x.transpose(2, 1, 0))  # [p, k, m]
        out[:, KCH * off : KCH * (off + sz)] = x.reshape(128, KCH * sz)
    return out


def kernel(reps, W, b, labels):
    from concourse.bass_utils import run_bass_kernel_spmd

    reps = np.asarray(reps, dtype=np.float32)
    W = np.asarray(W, dtype=np.float32)
    b = np.asarray(b, dtype=np.float32)
    labels_np = np.asarray(labels)

    if "nc" not in _CACHE:
        _CACHE["nc"] = _build_nc()
    nc = _CACHE["nc"]

    wq, uzw4, uzw64, bias4 = _prepare_static(W, b)
    consts_np = _pack_consts(uzw4, uzw64, wq, bias4)

    in_maps = []
    for core in range(NCORES):
        sh = slice(core * SHARD, (core + 1) * SHARD)
        repsq_np = np.concatenate(
            [consts_np, _prepare_reps(reps[sh]).view(np.uint8)], axis=1
        )
        in_maps.append(
            {
                "repsq": repsq_np,
                "maskc": _prepare_mask(labels_np[sh]),
            }
        )

    trace = bool(int(os.environ.get("CC_KERNEL_TRACE", "0")))
    res = run_bass_kernel_spmd(
        nc, in_maps, core_ids=list(range(NCORES)), trace=trace
    )
    if trace:
        _CACHE["last_results"] = res

    total = np.float64(0.0)
    for core in range(NCORES):
        total += np.float64(res.results[core]["partials"].sum(dtype=np.float64))
    loss = -(total / B)
    return np.float32(loss)



# revision 37
# speedup vs baseline: 1.1095x; 1.1095x over previous
"""Data-parallel Trainium2 kernel for the weighted classification loss.

loss = -mean_b sum_c w[b,c] * log(1 - softmax(reps @ W.T + b)[b,c])

Strategy (8 cores, batch-sharded 4096 rows each):
  - Host pre-casts reps to fp8e4 and pre-transposes into a matmul-ready
    [k-chunk x sample] layout; the kernel streams it HBM->SBUF with
    plain HWDGE DMAs (no on-chip cast/transpose).
  - Tapered chunks ([1024,1024,1024,512,256,256] samples): each chunk
    is an independent round whose matmuls start on its own DMA
    semaphore; the small final chunks shrink the post-stream tail.
  - Main matmul per chunk: K=128 fp8 chains over 8 D-chunks, 4-way
    column-tiled (tile_position=(0,32j), k-outer/j-inner) so 4
    sample-quarters accumulate concurrently into one PSUM tile as
    logits rows 32j..32j+9.
  - exp(l + bias) on ACT over the whole [128, Q] tile (4 groups at
    once); one diagonal-packed matmul vs a (ones - I | ones)-style
    stationary computes u_c = den - e_c (sum of positives) and den for
    all 4 groups; Ln on ACT; a host-prepared per-sample weight mask
    {0,1,2,-14} contracts w * ln(u) - 14*ln(den) via one DVE
    scalar_tensor_tensor with free-dim accumulate per chunk.
  - Per-chunk partial-sum columns DMA out as they finish (only the
    last sits on the critical tail); host combines.
"""

import os
import sys

import numpy as np

if "/opt/trn_rl_repo" not in sys.path:
    sys.path.insert(0, "/opt/trn_rl_repo")

import ml_dtypes

B, D, C = 32768, 1024, 10
NCORES = 8
SHARD = B // NCORES  # 4096
KCH = D // 128       # 8 contraction chunks
NGRP = 4
# chunk sizes in samples; each is one PSUM round of 4 column-groups.
# The PE FIFO costs ~27ns/instruction, so a 4-group chunk below ~900
# samples is issue-bound (~1.7us floor). Big data-bound chunks minimize
# total PE work; the small LAST chunk runs 2 column-groups (q=128,
# data-bound ~1.1us) to shorten the post-stream tail chain.
# chunk0 is sized so consts(276B) + KCH*s0 stays <= one 8192B packet per
# line; the tiny 2-group last chunk minimizes the dependency chain gated
# on the final (straggler-exposed) stretch of the stream.
CHUNKS = [960, 1024, 1024, 960, 128]
GRPS = [4, 4, 4, 4, 2]  # column groups per chunk (128//g row stride)
assert sum(CHUNKS) == SHARD and all(
    s % g == 0 for s, g in zip(CHUNKS, GRPS)
)
OFFS = [sum(CHUNKS[:i]) for i in range(len(CHUNKS))]
NCHK = len(CHUNKS)
# mask column offsets: chunk r occupies CHUNKS[r]//GRPS[r] mask columns
MOFFS = [sum(s // g for s, g in zip(CHUNKS[:i], GRPS[:i]))
         for i in range(NCHK)]
MASKW = sum(s // g for s, g in zip(CHUNKS, GRPS))
MID = 5
OPP_W = 2.0

_CACHE: dict = {}


def _build_nc():
    from contextlib import ExitStack

    import concourse.mybir as mybir
    import concourse.tile as tile
    from concourse import bacc
    from concourse.tile import add_dep_helper

    f32 = mybir.dt.float32
    bf16 = mybir.dt.bfloat16
    fp8 = mybir.dt.float8e4
    Exp = mybir.ActivationFunctionType.Exp
    Ln = mybir.ActivationFunctionType.Ln
    alu = mybir.AluOpType

    nc = bacc.Bacc(
        "TRN2",
        target_bir_lowering=False,
        debug=False,
        enable_asserts=False,
        num_devices=NCORES,
    )
    u8 = mybir.dt.uint8
    # The small consts block (uzw4 | uzw64 | wq | bias) is PREPENDED to
    # chunk0 inside the repsq tensor, so one DMA (one ~0.7us SP issue, one
    # semaphore) carries consts+chunk0 — all later chunk issues and
    # completions shift earlier by an issue slot. The big per-sample
    # weight mask rides BEHIND the last input chunk as its own DMA: every
    # chunk's completion shifts earlier by the mask's stream time, and
    # the mask still lands before the STTs need it (they trail the last
    # chunk's matmul+EXP+LN chain anyway).
    UZW_B = 2 * (32 + 64)
    WQ_B = KCH * C
    CONST_B = UZW_B + WQ_B + 4
    MASK_B = 2 * MASKW
    repsq = nc.dram_tensor("repsq", [128, CONST_B + KCH * SHARD], u8,
                           kind="ExternalInput").ap()
    maskc = nc.dram_tensor("maskc", [128, MASK_B], u8,
                           kind="ExternalInput").ap()
    partials = nc.dram_tensor("partials", [128, NCHK], f32,
                              kind="ExternalOutput").ap()

    with tile.TileContext(nc) as tc:
        with ExitStack() as ctx:
            const_pool = ctx.enter_context(tc.tile_pool(name="const", bufs=1))
            sb_pool = ctx.enter_context(tc.tile_pool(name="sb", bufs=3))
            lp_pool = ctx.enter_context(
                tc.tile_pool(name="lp", bufs=3, space="PSUM"))
            u_pool = ctx.enter_context(
                tc.tile_pool(name="u", bufs=2, space="PSUM"))

            # Pin the combined exp+ln activation table (set 6:
            # natural_log_exp_and_others) once, up front, so the compiler's
            # per-function table placement doesn't ping-pong reloads.
            ld_tab = nc.scalar.add_instruction(
                mybir.InstLoadActFuncSet(
                    name=nc.get_next_instruction_name(),
                    ins=[],
                    outs=[],
                    act_func_set_id=6,
                )
            )

            # input chunks on the SP ring (FIFO => chunk c completes at
            # ~its share of the stream); chunk0's DMA also carries the
            # small consts block as a byte prefix. Per-chunk tiles keep
            # Tile's DMA->matmul deps per-chunk.
            xb = []
            cn_t = None
            for c, (off, sz) in enumerate(zip(OFFS, CHUNKS)):
                pre = CONST_B if c == 0 else 0
                t = const_pool.tile([128, pre + KCH * sz], u8, tag=f"x{c}")
                nc.sync.dma_start(
                    t[:],
                    repsq[:, CONST_B + KCH * off - pre
                          : CONST_B + KCH * (off + sz)],
                )
                if c == 0:
                    cn_t = t
                xb.append(
                    t[:, pre:].bitcast(fp8).rearrange(
                        "p (k m) -> p k m", k=KCH)
                )

            # the big weight mask streams BEHIND the last input chunk
            mk_t = const_pool.tile([128, MASK_B], u8, tag="mask")
            nc.sync.dma_start(mk_t[:], maskc)

            uz_all = cn_t[:, 0:UZW_B].bitcast(bf16)
            uzw_t = uz_all[:, 0:32]
            uzw64_t = uz_all[:, 32:96]
            wq_t = cn_t[:, UZW_B : UZW_B + WQ_B].bitcast(fp8)
            bias_t = cn_t[:, UZW_B + WQ_B : CONST_B].bitcast(f32)
            mask_t = mk_t[:].bitcast(bf16)
            acc = const_pool.tile([128, NCHK], f32, tag="acc")
            wv = wq_t.rearrange("p (k c) -> p k c", k=KCH)

            lp_tiles = {}
            first_act = None

            def emit_mains(r):
                g = GRPS[r]
                gw = 128 // g
                q = CHUNKS[r] // g
                lp = lp_pool.tile([128, q], f32, tag="lp", name=f"lp{r}")
                lp_tiles[r] = lp
                # k-outer / j-inner: adjacent MMs hit different col-groups
                # so all g stream concurrently (MATMUL issue is strict FIFO)
                for k in range(KCH):
                    for j in range(g):
                        nc.tensor.matmul(
                            lp[gw * j : gw * j + C, :],
                            wv[:, k, :],
                            xb[r][:, k, j * q : (j + 1) * q],
                            start=(k == 0),
                            stop=(k == KCH - 1),
                            skip_group_check=True,
                            tile_position=(0, gw * j),
                        )

            def emit_tail(r):
                nonlocal first_act
                g = GRPS[r]
                gw = 128 // g
                q = CHUNKS[r] // g
                moff = MOFFS[r]
                lp = lp_tiles.pop(r)
                e = sb_pool.tile([128, q], bf16, tag="e", name=f"e{r}")
                act = nc.scalar.activation(
                    e[:], lp[:], Exp, bias=bias_t, scale=1.0
                )
                if first_act is None:
                    first_act = act
                    add_dep_helper(
                        act.ins, ld_tab.ins, sync=False,
                        reason="combined exp+ln table pinned before first ACT",
                    )

                # u covers ALL 128 partitions (gw-wide stationary) so the
                # Ln below never reads stale PSUM (0 * NaN = NaN in STT)
                u = u_pool.tile([128, q], f32, tag="u", name=f"u{r}")
                uz = uzw_t if g == 4 else uzw64_t
                for j in range(g):
                    nc.tensor.matmul(
                        u[gw * j : gw * j + gw, :],
                        uz[gw * j : gw * j + C, :],
                        e[gw * j : gw * j + C, :],
                        start=True,
                        stop=True,
                        skip_group_check=True,
                        tile_position=(gw * j, gw * j),
                    )

                lnu = sb_pool.tile([128, q], bf16, tag="lnu", name=f"ln{r}")
                nc.scalar.activation(lnu[:], u[:], Ln)

                scr = sb_pool.tile([128, q], f32, tag="scr", name=f"sc{r}")
                nc.vector.scalar_tensor_tensor(
                    out=scr[:],
                    in0=mask_t[:, moff : moff + q],
                    scalar=1.0,
                    in1=lnu[:],
                    op0=alu.mult,
                    op1=alu.mult,
                    accum_out=acc[:, r : r + 1],
                )

            # software-pipelined: round r's tail is emitted after round
            # r+1's matmuls so the PE never stalls waiting on ACT
            emit_mains(0)
            for r in range(1, NCHK):
                emit_mains(r)
                emit_tail(r - 1)
            emit_tail(NCHK - 1)

            # one output DMA for the whole accumulator: splitting it in two
            # backfires — the SP engine is still mid-issue on the first
            # when the last column's semaphore fires, so the single DMA's
            # issue actually starts (and lands) earlier
            nc.sync.dma_start(partials, acc[:])

    nc.compile()
    return nc


def _prepare_static(W: np.ndarray, b: np.ndarray):
    # wq[p, k*C + c] = fp8(W[c, 128k + p])
    wq = np.zeros((128, KCH * C), dtype=np.float32)
    for k in range(KCH):
        wq[:, k * C : (k + 1) * C] = W[:, k * 128 : (k + 1) * 128].T
    wq = wq.astype(ml_dtypes.float8_e4m3)

    # u = uzw_ext.T @ e per group: cols 0..9 -> den - e_c (sum of
    # positives), cols 10.. -> den (keeps every PSUM row defined > 0)
    def uzw_block(width):
        ext = np.ones((C, width), dtype=np.float32)
        ext[:, :C] -= np.eye(C, dtype=np.float32)
        blk = np.zeros((128, width), dtype=np.float32)
        for j in range(128 // width):
            blk[width * j : width * j + C, :] = ext
        return blk

    uzw4 = uzw_block(32)
    uzw64 = uzw_block(64)

    bias4 = np.zeros((128, 1), dtype=np.float32)
    for j in range(NGRP):
        bias4[32 * j : 32 * j + C, 0] = b
    return wq, uzw4, uzw64, bias4


def _prepare_mask(labels_sh: np.ndarray) -> np.ndarray:
    """Per-sample weight mask, bf16 [128, MASKW] viewed as uint8. Layout
    mirrors the on-chip per-chunk groups: chunk r, group j, n -> sample
    OFFS[r] + j*q + n at mask[gw*j + c, MOFFS[r] + n]."""
    cc = np.arange(C).reshape(1, C)
    m = np.zeros((128, MASKW), dtype=np.float32)
    for r, (off, sz) in enumerate(zip(OFFS, CHUNKS)):
        g = GRPS[r]
        gw = 128 // g
        q = sz // g
        moff = MOFFS[r]
        for j in range(g):
            lab = labels_sh[off + j * q : off + (j + 1) * q].astype(np.int64)
            ll = lab.reshape(q, 1)
            opp = (cc < MID) != (ll < MID)
            w = np.where(cc == ll, 0.0, np.where(opp, OPP_W, 1.0))  # [q, C]
            m[gw * j : gw * j + C, moff : moff + q] = w.T
            m[gw * j + C, moff : moff + q] = -float(C + MID - 1)
    return np.ascontiguousarray(m.astype(ml_dtypes.bfloat16)).view(np.uint8)


def _pack_consts(uzw4, uzw64, wq_fp8, bias4_f32) -> np.ndarray:
    """One [128, UZW_B+WQ_B+4] uint8 tensor: uzw4 | uzw64 | wq | bias4."""
    out = np.concatenate(
        [
            np.ascontiguousarray(uzw4.astype(ml_dtypes.bfloat16)).view(
                np.uint8),
            np.ascontiguousarray(uzw64.astype(ml_dtypes.bfloat16)).view(
                np.uint8),
            np.ascontiguousarray(wq_fp8).view(np.uint8),
            np.ascontiguousarray(bias4_f32).view(np.uint8),
        ],
        axis=1,
    )
    return out


def _prepare_reps(reps_sh: np.ndarray) -> np.ndarray:
    """repsq[p, KCH*off + k*sz + m] = fp8(reps_sh[off + m, 128k + p])
    for each chunk (off, sz)."""
    out = np.empty((128, KCH * SHARD), dtype=ml_dtypes.float8_e4m3)
    for off, sz in zip(OFFS, CHUNKS):
        x = reps_sh[off : off + sz].astype(ml_dtypes.float8_e4m3)
        x = x.reshape(sz, KCH, 128)                 # [m, k, p]
        x = np.ascontiguousarray(x.transpose(2, 1, 0))  # [p, k, m]
        out[:, KCH * off : KCH * (off + sz)] = x.reshape(128, KCH * sz)
    return out


def kernel(reps, W, b, labels):
    from concourse.bass_utils import run_bass_kernel_spmd

    reps = np.asarray(reps, dtype=np.float32)
    W = np.asarray(W, dtype=np.float32)
    b = np.asarray(b, dtype=np.float32)
    labels_np = np.asarray(labels)

    if "nc" not in _CACHE:
        _CACHE["nc"] = _build_nc()
    nc = _CACHE["nc"]

    wq, uzw4, uzw64, bias4 = _prepare_static(W, b)
    consts_np = _pack_consts(uzw4, uzw64, wq, bias4)

    in_maps = []
    for core in range(NCORES):
        sh = slice(core * SHARD, (core + 1) * SHARD)
        repsq_np = np.concatenate(
            [consts_np, _prepare_reps(reps[sh]).view(np.uint8)], axis=1
        )
        in_maps.append(
            {
                "repsq": repsq_np,
                "maskc": _prepare_mask(labels_np[sh]),
            }
        )

    trace = bool(int(os.environ.get("CC_KERNEL_TRACE", "0")))
    res = run_bass_kernel_spmd(
        nc, in_maps, core_ids=list(range(NCORES)), trace=trace
    )
    if trace:
        _CACHE["last_results"] = res

    total = np.float64(0.0)
    for core in range(NCORES):
        total += np.float64(res.results[core]["partials"].sum(dtype=np.float64))
    loss = -(total / B)
    return np.float32(loss)



# revision 41
# speedup vs baseline: 1.1179x; 1.0076x over previous
"""Data-parallel Trainium2 kernel for the weighted classification loss.

loss = -mean_b sum_c w[b,c] * log(1 - softmax(reps @ W.T + b)[b,c])

Strategy (8 cores, batch-sharded 4096 rows each):
  - Host pre-casts reps to fp8e4 and pre-transposes into a matmul-ready
    [k-chunk x sample] layout; the kernel streams it HBM->SBUF with
    plain HWDGE DMAs (no on-chip cast/transpose).
  - Tapered chunks ([1024,1024,1024,512,256,256] samples): each chunk
    is an independent round whose matmuls start on its own DMA
    semaphore; the small final chunks shrink the post-stream tail.
  - Main matmul per chunk: K=128 fp8 chains over 8 D-chunks, 4-way
    column-tiled (tile_position=(0,32j), k-outer/j-inner) so 4
    sample-quarters accumulate concurrently into one PSUM tile as
    logits rows 32j..32j+9.
  - exp(l + bias) on ACT over the whole [128, Q] tile (4 groups at
    once); one diagonal-packed matmul vs a (ones - I | ones)-style
    stationary computes u_c = den - e_c (sum of positives) and den for
    all 4 groups; Ln on ACT; a host-prepared per-sample weight mask
    {0,1,2,-14} contracts w * ln(u) - 14*ln(den) via one DVE
    scalar_tensor_tensor with free-dim accumulate per chunk.
  - Per-chunk partial-sum columns DMA out as they finish (only the
    last sits on the critical tail); host combines.
"""

import os
import sys

import numpy as np

if "/opt/trn_rl_repo" not in sys.path:
    sys.path.insert(0, "/opt/trn_rl_repo")

import ml_dtypes

B, D, C = 32768, 1024, 10
NCORES = 8
SHARD = B // NCORES  # 4096
KCH = D // 128       # 8 contraction chunks
NGRP = 4
# chunk sizes in samples; each is one PSUM round of 4 column-groups.
# The PE FIFO costs ~27ns/instruction, so a 4-group chunk below ~900
# samples is issue-bound (~1.7us floor). Big data-bound chunks minimize
# total PE work; the small LAST chunk runs 2 column-groups (q=128,
# data-bound ~1.1us) to shorten the post-stream tail chain.
# chunk0 is sized so consts(276B) + KCH*s0 stays <= one 8192B packet per
# line; the tiny 2-group last chunk minimizes the dependency chain gated
# on the final (straggler-exposed) stretch of the stream.
CHUNKS = [960, 1024, 1024, 960, 128]
GRPS = [4, 4, 4, 4, 2]  # column groups per chunk (128//g row stride)
assert sum(CHUNKS) == SHARD and all(
    s % g == 0 for s, g in zip(CHUNKS, GRPS)
)
OFFS = [sum(CHUNKS[:i]) for i in range(len(CHUNKS))]
NCHK = len(CHUNKS)
# mask column offsets: chunk r occupies CHUNKS[r]//GRPS[r] mask columns
MOFFS = [sum(s // g for s, g in zip(CHUNKS[:i], GRPS[:i]))
         for i in range(NCHK)]
MASKW = sum(s // g for s, g in zip(CHUNKS, GRPS))
MID = 5
OPP_W = 2.0

_CACHE: dict = {}


def _build_nc():
    from contextlib import ExitStack

    import concourse.mybir as mybir
    import concourse.tile as tile
    from concourse import bacc
    from concourse.tile import add_dep_helper

    f32 = mybir.dt.float32
    bf16 = mybir.dt.bfloat16
    fp8 = mybir.dt.float8e4
    Exp = mybir.ActivationFunctionType.Exp
    Ln = mybir.ActivationFunctionType.Ln
    alu = mybir.AluOpType

    nc = bacc.Bacc(
        "TRN2",
        target_bir_lowering=False,
        debug=False,
        enable_asserts=False,
        num_devices=NCORES,
    )
    u8 = mybir.dt.uint8
    # The small consts block (uzw4 | uzw64 | wq | bias) is PREPENDED to
    # chunk0 inside the repsq tensor, so one DMA (one ~0.7us SP issue, one
    # semaphore) carries consts+chunk0 — all later chunk issues and
    # completions shift earlier by an issue slot. The big per-sample
    # weight mask rides BEHIND the last input chunk as its own DMA: every
    # chunk's completion shifts earlier by the mask's stream time, and
    # the mask still lands before the STTs need it (they trail the last
    # chunk's matmul+EXP+LN chain anyway).
    UZW_B = 2 * (32 + 64)
    WQ_B = KCH * C
    CONST_B = UZW_B + WQ_B + 4
    MASK_B = 2 * MASKW
    repsq = nc.dram_tensor("repsq", [128, CONST_B + KCH * SHARD], u8,
                           kind="ExternalInput").ap()
    # only rows {32g + 0..10} of the mask are nonzero (the 2-group last
    # chunk's rows {64j + 0..10} are a subset), so the DMA moves just 44
    # lines; the zero rows are memset once on-chip
    maskc = nc.dram_tensor("maskc", [44, MASK_B], u8,
                           kind="ExternalInput").ap()
    partials = nc.dram_tensor("partials", [128, NCHK], f32,
                              kind="ExternalOutput").ap()

    with tile.TileContext(nc) as tc:
        with ExitStack() as ctx:
            const_pool = ctx.enter_context(tc.tile_pool(name="const", bufs=1))
            sb_pool = ctx.enter_context(tc.tile_pool(name="sb", bufs=3))
            lp_pool = ctx.enter_context(
                tc.tile_pool(name="lp", bufs=3, space="PSUM"))
            u_pool = ctx.enter_context(
                tc.tile_pool(name="u", bufs=2, space="PSUM"))

            # Pin the combined exp+ln activation table (set 6:
            # natural_log_exp_and_others) once, up front, so the compiler's
            # per-function table placement doesn't ping-pong reloads.
            ld_tab = nc.scalar.add_instruction(
                mybir.InstLoadActFuncSet(
                    name=nc.get_next_instruction_name(),
                    ins=[],
                    outs=[],
                    act_func_set_id=6,
                )
            )

            # input chunks on the SP ring (FIFO => chunk c completes at
            # ~its share of the stream); chunk0's DMA also carries the
            # small consts block as a byte prefix. Per-chunk tiles keep
            # Tile's DMA->matmul deps per-chunk.
            xb = []
            cn_t = None
            for c, (off, sz) in enumerate(zip(OFFS, CHUNKS)):
                pre = CONST_B if c == 0 else 0
                t = const_pool.tile([128, pre + KCH * sz], u8, tag=f"x{c}")
                nc.sync.dma_start(
                    t[:],
                    repsq[:, CONST_B + KCH * off - pre
                          : CONST_B + KCH * (off + sz)],
                )
                if c == 0:
                    cn_t = t
                xb.append(
                    t[:, pre:].bitcast(fp8).rearrange(
                        "p (k m) -> p k m", k=KCH)
                )

            # the weight mask streams BEHIND the last input chunk; only
            # the 44 nonzero rows are transferred (4 DMAs of 11 contiguous
            # partitions), the rest is zeroed once by the idle Pool engine
            mk_t = const_pool.tile([128, MASK_B], u8, tag="mask")
            nc.gpsimd.memset(mk_t[:], 0)
            for g4 in range(4):
                nc.sync.dma_start(
                    mk_t[32 * g4 : 32 * g4 + 11, :],
                    maskc[11 * g4 : 11 * g4 + 11, :],
                )

            uz_all = cn_t[:, 0:UZW_B].bitcast(bf16)
            uzw_t = uz_all[:, 0:32]
            uzw64_t = uz_all[:, 32:96]
            wq_t = cn_t[:, UZW_B : UZW_B + WQ_B].bitcast(fp8)
            bias_t = cn_t[:, UZW_B + WQ_B : CONST_B].bitcast(f32)
            mask_t = mk_t[:].bitcast(bf16)
            acc = const_pool.tile([128, NCHK], f32, tag="acc")
            wv = wq_t.rearrange("p (k c) -> p k c", k=KCH)

            lp_tiles = {}
            first_act = None

            def emit_mains(r):
                g = GRPS[r]
                gw = 128 // g
                q = CHUNKS[r] // g
                lp = lp_pool.tile([128, q], f32, tag="lp", name=f"lp{r}")
                lp_tiles[r] = lp
                # k-outer / j-inner: adjacent MMs hit different col-groups
                # so all g stream concurrently (MATMUL issue is strict FIFO)
                for k in range(KCH):
                    for j in range(g):
                        nc.tensor.matmul(
                            lp[gw * j : gw * j + C, :],
                            wv[:, k, :],
                            xb[r][:, k, j * q : (j + 1) * q],
                            start=(k == 0),
                            stop=(k == KCH - 1),
                            skip_group_check=True,
                            tile_position=(0, gw * j),
                        )

            def emit_tail(r):
                nonlocal first_act
                g = GRPS[r]
                gw = 128 // g
                q = CHUNKS[r] // g
                moff = MOFFS[r]
                lp = lp_tiles.pop(r)
                e = sb_pool.tile([128, q], bf16, tag="e", name=f"e{r}")
                act = nc.scalar.activation(
                    e[:], lp[:], Exp, bias=bias_t, scale=1.0
                )
                if first_act is None:
                    first_act = act
                    add_dep_helper(
                        act.ins, ld_tab.ins, sync=False,
                        reason="combined exp+ln table pinned before first ACT",
                    )

                # u covers ALL 128 partitions (gw-wide stationary) so the
                # Ln below never reads stale PSUM (0 * NaN = NaN in STT)
                u = u_pool.tile([128, q], f32, tag="u", name=f"u{r}")
                uz = uzw_t if g == 4 else uzw64_t
                for j in range(g):
                    nc.tensor.matmul(
                        u[gw * j : gw * j + gw, :],
                        uz[gw * j : gw * j + C, :],
                        e[gw * j : gw * j + C, :],
                        start=True,
                        stop=True,
                        skip_group_check=True,
                        tile_position=(gw * j, gw * j),
                    )

                lnu = sb_pool.tile([128, q], bf16, tag="lnu", name=f"ln{r}")
                nc.scalar.activation(lnu[:], u[:], Ln)

                scr = sb_pool.tile([128, q], f32, tag="scr", name=f"sc{r}")
                nc.vector.scalar_tensor_tensor(
                    out=scr[:],
                    in0=mask_t[:, moff : moff + q],
                    scalar=1.0,
                    in1=lnu[:],
                    op0=alu.mult,
                    op1=alu.mult,
                    accum_out=acc[:, r : r + 1],
                )

            # software-pipelined: round r's tail is emitted after round
            # r+1's matmuls so the PE never stalls waiting on ACT
            emit_mains(0)
            for r in range(1, NCHK):
                emit_mains(r)
                emit_tail(r - 1)
            emit_tail(NCHK - 1)

            # one output DMA for the whole accumulator: splitting it in two
            # backfires — the SP engine is still mid-issue on the first
            # when the last column's semaphore fires, so the single DMA's
            # issue actually starts (and lands) earlier
            nc.sync.dma_start(partials, acc[:])

    nc.compile()
    return nc


def _prepare_static(W: np.ndarray, b: np.ndarray):
    # wq[p, k*C + c] = fp8(W[c, 128k + p])
    wq = np.zeros((128, KCH * C), dtype=np.float32)
    for k in range(KCH):
        wq[:, k * C : (k + 1) * C] = W[:, k * 128 : (k + 1) * 128].T
    wq = wq.astype(ml_dtypes.float8_e4m3)

    # u = uzw_ext.T @ e per group: cols 0..9 -> den - e_c (sum of
    # positives), cols 10.. -> den (keeps every PSUM row defined > 0)
    def uzw_block(width):
        ext = np.ones((C, width), dtype=np.float32)
        ext[:, :C] -= np.eye(C, dtype=np.float32)
        blk = np.zeros((128, width), dtype=np.float32)
        for j in range(128 // width):
            blk[width * j : width * j + C, :] = ext
        return blk

    uzw4 = uzw_block(32)
    uzw64 = uzw_block(64)

    bias4 = np.zeros((128, 1), dtype=np.float32)
    for j in range(NGRP):
        bias4[32 * j : 32 * j + C, 0] = b
    return wq, uzw4, uzw64, bias4


def _prepare_mask(labels_sh: np.ndarray) -> np.ndarray:
    """Per-sample weight mask, bf16 [44, MASKW] viewed as uint8: row
    11*g + rr carries on-chip partition 32*g + rr. Layout mirrors the
    on-chip per-chunk groups: chunk r, group j, n -> sample
    OFFS[r] + j*q + n at partition gw*j + c, column MOFFS[r] + n."""
    cc = np.arange(C).reshape(1, C)
    m = np.zeros((128, MASKW), dtype=np.float32)
    for r, (off, sz) in enumerate(zip(OFFS, CHUNKS)):
        g = GRPS[r]
        gw = 128 // g
        q = sz // g
        moff = MOFFS[r]
        for j in range(g):
            lab = labels_sh[off + j * q : off + (j + 1) * q].astype(np.int64)
            ll = lab.reshape(q, 1)
            opp = (cc < MID) != (ll < MID)
            w = np.where(cc == ll, 0.0, np.where(opp, OPP_W, 1.0))  # [q, C]
            m[gw * j : gw * j + C, moff : moff + q] = w.T
            m[gw * j + C, moff : moff + q] = -float(C + MID - 1)
    m44 = np.concatenate([m[32 * g : 32 * g + 11, :] for g in range(4)])
    return np.ascontiguousarray(
        m44.astype(ml_dtypes.bfloat16)).view(np.uint8)


def _pack_consts(uzw4, uzw64, wq_fp8, bias4_f32) -> np.ndarray:
    """One [128, UZW_B+WQ_B+4] uint8 tensor: uzw4 | uzw64 | wq | bias4."""
    out = np.concatenate(
        [
            np.ascontiguousarray(uzw4.astype(ml_dtypes.bfloat16)).view(
                np.uint8),
            np.ascontiguousarray(uzw64.astype(ml_dtypes.bfloat16)).view(
                np.uint8),
            np.ascontiguousarray(wq_fp8).view(np.uint8),
            np.ascontiguousarray(bias4_f32).view(np.uint8),
        ],
        axis=1,
    )
    return out


def _prepare_reps(reps_sh: np.ndarray) -> np.ndarray:
    """repsq[p, KCH*off + k*sz + m] = fp8(reps_sh[off + m, 128k + p])
    for each chunk (off, sz)."""
    out = np.empty((128, KCH * SHARD), dtype=ml_dtypes.float8_e4m3)
    for off, sz in zip(OFFS, CHUNKS):
        x = reps_sh[off : off + sz].astype(ml_dtypes.float8_e4m3)
        x = x.reshape(sz, KCH, 128)                 # [m, k, p]
        x = np.ascontiguousarray(x.transpose(2, 1, 0))  # [p, k, m]
        out[:, KCH * off : KCH * (off + sz)] = x.reshape(128, KCH * sz)
    return out


def kernel(reps, W, b, labels):
    from concourse.bass_utils import run_bass_kernel_spmd

    reps = np.asarray(reps, dtype=np.float32)
    W = np.asarray(W, dtype=np.float32)
    b = np.asarray(b, dtype=np.float32)
    labels_np = np.asarray(labels)

    if "nc" not in _CACHE:
        _CACHE["nc"] = _build_nc()
    nc = _CACHE["nc"]

    wq, uzw4, uzw64, bias4 = _prepare_static(W, b)
    consts_np = _pack_consts(uzw4, uzw64, wq, bias4)

    in_maps = []
    for core in range(NCORES):
        sh = slice(core * SHARD, (core + 1) * SHARD)
        repsq_np = np.concatenate(
            [consts_np, _prepare_reps(reps[sh]).view(np.uint8)], axis=1
        )
        in_maps.append(
            {
                "repsq": repsq_np,
                "maskc": _prepare_mask(labels_np[sh]),
            }
        )

    trace = bool(int(os.environ.get("CC_KERNEL_TRACE", "0")))
    res = run_bass_kernel_spmd(
        nc, in_maps, core_ids=list(range(NCORES)), trace=trace
    )
    if trace:
        _CACHE["last_results"] = res

    total = np.float64(0.0)
    for core in range(NCORES):
        total += np.float64(res.results[core]["partials"].sum(dtype=np.float64))
    loss = -(total / B)
    return np.float32(loss)



# revision 44
# speedup vs baseline: 1.1270x; 1.0081x over previous
"""Data-parallel Trainium2 kernel for the weighted classification loss.

loss = -mean_b sum_c w[b,c] * log(1 - softmax(reps @ W.T + b)[b,c])

Strategy (8 cores, batch-sharded 4096 rows each):
  - Host pre-casts reps to fp8e4 and pre-transposes into a matmul-ready
    [k-chunk x sample] layout; the kernel streams it HBM->SBUF on one
    HWDGE ring (no on-chip cast/transpose). The small consts (uzw | wq |
    bias) ride as a byte prefix of chunk0's DMA; the per-sample weight
    mask (44 nonzero rows only, rest memset to 0) rides BEHIND the last
    chunk so every chunk's completion comes earlier and only the short
    final STT chain can be gated on it.
  - Chunks [960,1024,1024,960,128]: per-chunk DMA semaphores gate that
    chunk's matmuls. The PE FIFO costs ~27ns/instruction, so 4-group
    chunks below ~900 samples are issue-bound — big data-bound chunks
    minimize total PE work, while the tiny 2-group last chunk minimizes
    the dependency chain exposed to the slowest DMA engine (the ring
    owner intermittently runs ~2x slow, adding 1-4us to stream end).
  - Main matmul per chunk: K=128 fp8 chains over 8 D-chunks, g-way
    column-tiled (tile_position=(0,(128/g)j), k-outer/j-inner) so all g
    sample-groups accumulate concurrently into one PSUM tile as logits
    rows (128/g)j..+9.
  - exp(l + bias) on ACT over the whole [128, Q] tile (all groups at
    once); one diagonal-packed matmul vs a (ones - I | ones)-style
    stationary computes u_c = den - e_c (sum of positives) and den for
    all groups across ALL 128 partitions (no stale-PSUM Ln); Ln on ACT;
    the host-prepared mask {0,1,2,-14} contracts w * ln(u) - 14*ln(den)
    via one DVE scalar_tensor_tensor with free-dim accumulate per chunk.
  - One [128, NCHK] partials DMA at the end; host combines.
"""

import os
import sys

import numpy as np

if "/opt/trn_rl_repo" not in sys.path:
    sys.path.insert(0, "/opt/trn_rl_repo")

import ml_dtypes

B, D, C = 32768, 1024, 10
NCORES = 8
SHARD = B // NCORES  # 4096
KCH = D // 128       # 8 contraction chunks
NGRP = 4
# chunk sizes in samples; each is one PSUM round of 4 column-groups.
# The PE FIFO costs ~27ns/instruction, so a 4-group chunk below ~900
# samples is issue-bound (~1.7us floor). Big data-bound chunks minimize
# total PE work; the small LAST chunk runs 2 column-groups (q=128,
# data-bound ~1.1us) to shorten the post-stream tail chain.
# chunk0 is sized so consts(276B) + KCH*s0 stays <= one 8192B packet per
# line; the tiny 2-group last chunk minimizes the dependency chain gated
# on the final (straggler-exposed) stretch of the stream.
CHUNKS = [960, 1024, 1024, 960, 128]
GRPS = [4, 4, 4, 4, 2]  # column groups per chunk (128//g row stride)
assert sum(CHUNKS) == SHARD and all(
    s % g == 0 for s, g in zip(CHUNKS, GRPS)
)
OFFS = [sum(CHUNKS[:i]) for i in range(len(CHUNKS))]
NCHK = len(CHUNKS)
# mask column offsets: chunk r occupies CHUNKS[r]//GRPS[r] mask columns
MOFFS = [sum(s // g for s, g in zip(CHUNKS[:i], GRPS[:i]))
         for i in range(NCHK)]
MASKW = sum(s // g for s, g in zip(CHUNKS, GRPS))
MID = 5
OPP_W = 2.0

_CACHE: dict = {}


def _build_nc():
    from contextlib import ExitStack

    import concourse.mybir as mybir
    import concourse.tile as tile
    from concourse import bacc
    from concourse.tile import add_dep_helper

    f32 = mybir.dt.float32
    bf16 = mybir.dt.bfloat16
    fp8 = mybir.dt.float8e4
    Exp = mybir.ActivationFunctionType.Exp
    Ln = mybir.ActivationFunctionType.Ln
    alu = mybir.AluOpType

    nc = bacc.Bacc(
        "TRN2",
        target_bir_lowering=False,
        debug=False,
        enable_asserts=False,
        num_devices=NCORES,
    )
    u8 = mybir.dt.uint8
    # The small consts block (uzw4 | uzw64 | wq | bias) is PREPENDED to
    # chunk0 inside the repsq tensor, so one DMA (one ~0.7us SP issue, one
    # semaphore) carries consts+chunk0 — all later chunk issues and
    # completions shift earlier by an issue slot. The big per-sample
    # weight mask rides BEHIND the last input chunk as its own DMA: every
    # chunk's completion shifts earlier by the mask's stream time, and
    # the mask still lands before the STTs need it (they trail the last
    # chunk's matmul+EXP+LN chain anyway).
    UZW_B = 2 * (32 + 64)
    WQ_B = KCH * C
    CONST_B = UZW_B + WQ_B + 4
    MASK_B = 2 * MASKW
    repsq = nc.dram_tensor("repsq", [128, CONST_B + KCH * SHARD], u8,
                           kind="ExternalInput").ap()
    # only rows {32g + 0..10} of the mask are nonzero (the 2-group last
    # chunk's rows {64j + 0..10} are a subset), so the DMA moves just 44
    # lines; the zero rows are memset once on-chip
    maskc = nc.dram_tensor("maskc", [44, MASK_B], u8,
                           kind="ExternalInput").ap()
    partials = nc.dram_tensor("partials", [128, NCHK], f32,
                              kind="ExternalOutput").ap()

    with tile.TileContext(nc) as tc:
        with ExitStack() as ctx:
            const_pool = ctx.enter_context(tc.tile_pool(name="const", bufs=1))
            sb_pool = ctx.enter_context(tc.tile_pool(name="sb", bufs=3))
            lp_pool = ctx.enter_context(
                tc.tile_pool(name="lp", bufs=3, space="PSUM"))
            u_pool = ctx.enter_context(
                tc.tile_pool(name="u", bufs=2, space="PSUM"))

            # Pin the combined exp+ln activation table (set 6:
            # natural_log_exp_and_others) once, up front, so the compiler's
            # per-function table placement doesn't ping-pong reloads.
            ld_tab = nc.scalar.add_instruction(
                mybir.InstLoadActFuncSet(
                    name=nc.get_next_instruction_name(),
                    ins=[],
                    outs=[],
                    act_func_set_id=6,
                )
            )

            # input chunks on the SP ring (FIFO => chunk c completes at
            # ~its share of the stream); chunk0's DMA also carries the
            # small consts block as a byte prefix. Per-chunk tiles keep
            # Tile's DMA->matmul deps per-chunk.
            xb = []
            cn_t = None
            for c, (off, sz) in enumerate(zip(OFFS, CHUNKS)):
                pre = CONST_B if c == 0 else 0
                t = const_pool.tile([128, pre + KCH * sz], u8, tag=f"x{c}")
                nc.sync.dma_start(
                    t[:],
                    repsq[:, CONST_B + KCH * off - pre
                          : CONST_B + KCH * (off + sz)],
                )
                if c == 0:
                    cn_t = t
                xb.append(
                    t[:, pre:].bitcast(fp8).rearrange(
                        "p (k m) -> p k m", k=KCH)
                )

            # the weight mask streams BEHIND the last input chunk; only
            # the 44 nonzero rows are transferred (4 DMAs of 11 contiguous
            # partitions), the rest is zeroed once by the idle Pool engine
            mk_t = const_pool.tile([128, MASK_B], u8, tag="mask")
            nc.gpsimd.memset(mk_t[:], 0)
            for g4 in range(4):
                nc.sync.dma_start(
                    mk_t[32 * g4 : 32 * g4 + 11, :],
                    maskc[11 * g4 : 11 * g4 + 11, :],
                )

            uz_all = cn_t[:, 0:UZW_B].bitcast(bf16)
            uzw_t = uz_all[:, 0:32]
            uzw64_t = uz_all[:, 32:96]
            wq_t = cn_t[:, UZW_B : UZW_B + WQ_B].bitcast(fp8)
            bias_t = cn_t[:, UZW_B + WQ_B : CONST_B].bitcast(f32)
            mask_t = mk_t[:].bitcast(bf16)
            acc = const_pool.tile([128, NCHK], f32, tag="acc")
            wv = wq_t.rearrange("p (k c) -> p k c", k=KCH)

            lp_tiles = {}
            first_act = None

            def emit_mains(r):
                g = GRPS[r]
                gw = 128 // g
                q = CHUNKS[r] // g
                lp = lp_pool.tile([128, q], f32, tag="lp", name=f"lp{r}")
                lp_tiles[r] = lp
                # k-outer / j-inner: adjacent MMs hit different col-groups
                # so all g stream concurrently (MATMUL issue is strict FIFO)
                for k in range(KCH):
                    for j in range(g):
                        nc.tensor.matmul(
                            lp[gw * j : gw * j + C, :],
                            wv[:, k, :],
                            xb[r][:, k, j * q : (j + 1) * q],
                            start=(k == 0),
                            stop=(k == KCH - 1),
                            skip_group_check=True,
                            tile_position=(0, gw * j),
                        )

            def emit_tail(r):
                nonlocal first_act
                g = GRPS[r]
                gw = 128 // g
                q = CHUNKS[r] // g
                moff = MOFFS[r]
                lp = lp_tiles.pop(r)
                e = sb_pool.tile([128, q], bf16, tag="e", name=f"e{r}")
                act = nc.scalar.activation(
                    e[:], lp[:], Exp, bias=bias_t, scale=1.0
                )
                if first_act is None:
                    first_act = act
                    add_dep_helper(
                        act.ins, ld_tab.ins, sync=False,
                        reason="combined exp+ln table pinned before first ACT",
                    )

                # u covers ALL 128 partitions (gw-wide stationary) so the
                # Ln below never reads stale PSUM (0 * NaN = NaN in STT)
                u = u_pool.tile([128, q], f32, tag="u", name=f"u{r}")
                uz = uzw_t if g == 4 else uzw64_t
                for j in range(g):
                    nc.tensor.matmul(
                        u[gw * j : gw * j + gw, :],
                        uz[gw * j : gw * j + C, :],
                        e[gw * j : gw * j + C, :],
                        start=True,
                        stop=True,
                        skip_group_check=True,
                        tile_position=(gw * j, gw * j),
                    )

                lnu = sb_pool.tile([128, q], bf16, tag="lnu", name=f"ln{r}")
                nc.scalar.activation(lnu[:], u[:], Ln)

                scr = sb_pool.tile([128, q], f32, tag="scr", name=f"sc{r}")
                nc.vector.scalar_tensor_tensor(
                    out=scr[:],
                    in0=mask_t[:, moff : moff + q],
                    scalar=1.0,
                    in1=lnu[:],
                    op0=alu.mult,
                    op1=alu.mult,
                    accum_out=acc[:, r : r + 1],
                )

            # software-pipelined: round r's tail is emitted after round
            # r+1's matmuls so the PE never stalls waiting on ACT
            emit_mains(0)
            for r in range(1, NCHK):
                emit_mains(r)
                emit_tail(r - 1)
            emit_tail(NCHK - 1)

            # one output DMA for the whole accumulator: splitting it in two
            # backfires — the SP engine is still mid-issue on the first
            # when the last column's semaphore fires, so the single DMA's
            # issue actually starts (and lands) earlier
            nc.sync.dma_start(partials, acc[:])

    nc.compile()
    return nc


def _prepare_static(W: np.ndarray, b: np.ndarray):
    # wq[p, k*C + c] = fp8(W[c, 128k + p])
    wq = np.zeros((128, KCH * C), dtype=np.float32)
    for k in range(KCH):
        wq[:, k * C : (k + 1) * C] = W[:, k * 128 : (k + 1) * 128].T
    wq = wq.astype(ml_dtypes.float8_e4m3)

    # u = uzw_ext.T @ e per group: cols 0..9 -> den - e_c (sum of
    # positives), cols 10.. -> den (keeps every PSUM row defined > 0)
    def uzw_block(width):
        ext = np.ones((C, width), dtype=np.float32)
        ext[:, :C] -= np.eye(C, dtype=np.float32)
        blk = np.zeros((128, width), dtype=np.float32)
        for j in range(128 // width):
            blk[width * j : width * j + C, :] = ext
        return blk

    uzw4 = uzw_block(32)
    uzw64 = uzw_block(64)

    bias4 = np.zeros((128, 1), dtype=np.float32)
    for j in range(NGRP):
        bias4[32 * j : 32 * j + C, 0] = b
    return wq, uzw4, uzw64, bias4


def _prepare_mask(labels_sh: np.ndarray) -> np.ndarray:
    """Per-sample weight mask, bf16 [44, MASKW] viewed as uint8: row
    11*g + rr carries on-chip partition 32*g + rr. Layout mirrors the
    on-chip per-chunk groups: chunk r, group j, n -> sample
    OFFS[r] + j*q + n at partition gw*j + c, column MOFFS[r] + n."""
    cc = np.arange(C).reshape(1, C)
    m = np.zeros((128, MASKW), dtype=np.float32)
    for r, (off, sz) in enumerate(zip(OFFS, CHUNKS)):
        g = GRPS[r]
        gw = 128 // g
        q = sz // g
        moff = MOFFS[r]
        for j in range(g):
            lab = labels_sh[off + j * q : off + (j + 1) * q].astype(np.int64)
            ll = lab.reshape(q, 1)
            opp = (cc < MID) != (ll < MID)
            w = np.where(cc == ll, 0.0, np.where(opp, OPP_W, 1.0))  # [q, C]
            m[gw * j : gw * j + C, moff : moff + q] = w.T
            m[gw * j + C, moff : moff + q] = -float(C + MID - 1)
    m44 = np.concatenate([m[32 * g : 32 * g + 11, :] for g in range(4)])
    return np.ascontiguousarray(
        m44.astype(ml_dtypes.bfloat16)).view(np.uint8)


def _pack_consts(uzw4, uzw64, wq_fp8, bias4_f32) -> np.ndarray:
    """One [128, UZW_B+WQ_B+4] uint8 tensor: uzw4 | uzw64 | wq | bias4."""
    out = np.concatenate(
        [
            np.ascontiguousarray(uzw4.astype(ml_dtypes.bfloat16)).view(
                np.uint8),
            np.ascontiguousarray(uzw64.astype(ml_dtypes.bfloat16)).view(
                np.uint8),
            np.ascontiguousarray(wq_fp8).view(np.uint8),
            np.ascontiguousarray(bias4_f32).view(np.uint8),
        ],
        axis=1,
    )
    return out


def _prepare_reps(reps_sh: np.ndarray) -> np.ndarray:
    """repsq[p, KCH*off + k*sz + m] = fp8(reps_sh[off + m, 128k + p])
    for each chunk (off, sz)."""
    out = np.empty((128, KCH * SHARD), dtype=ml_dtypes.float8_e4m3)
    for off, sz in zip(OFFS, CHUNKS):
        x = reps_sh[off : off + sz].astype(ml_dtypes.float8_e4m3)
        x = x.reshape(sz, KCH, 128)                 # [m, k, p]
        x = np.ascontiguousarray(x.transpose(2, 1, 0))  # [p, k, m]
        out[:, KCH * off : KCH * (off + sz)] = x.reshape(128, KCH * sz)
    return out


def kernel(reps, W, b, labels):
    from concourse.bass_utils import run_bass_kernel_spmd

    reps = np.asarray(reps, dtype=np.float32)
    W = np.asarray(W, dtype=np.float32)
    b = np.asarray(b, dtype=np.float32)
    labels_np = np.asarray(labels)

    if "nc" not in _CACHE:
        _CACHE["nc"] = _build_nc()
    nc = _CACHE["nc"]

    wq, uzw4, uzw64, bias4 = _prepare_static(W, b)
    consts_np = _pack_consts(uzw4, uzw64, wq, bias4)

    in_maps = []
    for core in range(NCORES):
        sh = slice(core * SHARD, (core + 1) * SHARD)
        repsq_np = np.concatenate(
            [consts_np, _prepare_reps(reps[sh]).view(np.uint8)], axis=1
        )
        in_maps.append(
            {
                "repsq": repsq_np,
                "maskc": _prepare_mask(labels_np[sh]),
            }
        )

    trace = bool(int(os.environ.get("CC_KERNEL_TRACE", "0")))
    res = run_bass_kernel_spmd(
        nc, in_maps, core_ids=list(range(NCORES)), trace=trace
    )
    if trace:
        _CACHE["last_results"] = res

    total = np.float64(0.0)
    for core in range(NCORES):
        total += np.float64(res.results[core]["partials"].sum(dtype=np.float64))
    loss = -(total / B)
    return np.float32(loss)

